# revision 6
# baseline (speedup 1.0000x reference)
"""DynamicRNN (LSTM) Trainium2 kernel.

Strategy (8 NeuronCores, SPMD, zero cross-core communication):
  - Data-parallel: batch B=128 sharded 8 ways (BS=16 per core). LSTM weights
    replicated. The sequential T=512 time loop runs locally per shard.
  - Phase 1: zx = x @ Wx + b for all timesteps, batched as [128-row, 2048]
    matmuls (full PE utilization), written to a DRAM scratch laid out
    [T, BS, 4H] so phase 2 reads one contiguous 128 KB block per step.
  - Phase 2 (recurrence): per step, z = zx_t + h @ Wh via PSUM accumulation:
    identity-matmul initializes PSUM with zx_t, then 4 k-tile matmuls with
    stationary h^T [128,16] tiles and moving Wh [128,512] chunks.
    Gates on ACT (sigmoid/tanh straight from PSUM), cell update on DVE,
    h^T for the next step via 4 PE transposes + rounding copies.
  - All matmuls run in float32r (rounded fp32): 1 cycle/row at N>=512
    (4x faster than native fp32) with ~1e-4 relative rounding error.
    The cell state c stays in full fp32.
"""
import numpy as np
import concourse.bass as bass
import concourse.tile as tile
from concourse import bacc, mybir
from concourse.bass_utils import run_bass_kernel_spmd

B, T, D, H = 128, 512, 256, 512
NCORES = 8
BS = B // NCORES          # 16 batch rows per core
G4 = 4 * H                # 2048
KD = D // 128             # 2 k-tiles for x @ Wx
KH = H // 128             # 4 k-tiles for h @ Wh
NG = G4 // 512            # 4 n-chunks (PSUM bank = 512 fp32)
F32 = mybir.dt.float32
F32R = mybir.dt.float32r
AFT = mybir.ActivationFunctionType


def build(t_steps=T, zx_prefetch=6):
    tb = min(128, t_steps)          # phase-1 time-block (rows per chunk)
    assert t_steps % tb == 0
    nc = bacc.Bacc("TRN2", target_bir_lowering=False, debug=False,
                   num_devices=NCORES)

    x_d = nc.dram_tensor("x", [BS, t_steps, D], F32, kind="ExternalInput").ap()
    wx_d = nc.dram_tensor("wx", [D, G4], F32, kind="ExternalInput").ap()
    wh_d = nc.dram_tensor("wh", [H, G4], F32, kind="ExternalInput").ap()
    b_d = nc.dram_tensor("b", [1, G4], F32, kind="ExternalInput").ap()
    i16_d = nc.dram_tensor("i16", [16, 16], F32, kind="ExternalInput").ap()
    i128_d = nc.dram_tensor("i128", [128, 128], F32, kind="ExternalInput").ap()
    ones_d = nc.dram_tensor("ones", [1, 128], F32, kind="ExternalInput").ap()

    out_d = nc.dram_tensor("out", [BS, t_steps, H], F32, kind="ExternalOutput").ap()
    hto_d = nc.dram_tensor("hto", [BS, H], F32, kind="ExternalOutput").ap()
    cto_d = nc.dram_tensor("cto", [BS, H], F32, kind="ExternalOutput").ap()

    zx_d = nc.dram_tensor("zx_scratch", [t_steps, BS, G4], F32R).ap()

    with tile.TileContext(nc) as tc:
        # ---------------- constants / weights ----------------
        with tc.tile_pool(name="const", bufs=1) as cpool:
            i16_f = cpool.tile([16, 16], F32, tag="i16f")
            nc.sync.dma_start(out=i16_f[:], in_=i16_d[:])
            i16_r = cpool.tile([16, 16], F32R, tag="i16r")
            nc.vector.tensor_copy(i16_r[:], i16_f[:])
            i128_f = cpool.tile([128, 128], F32, tag="i128f")
            nc.sync.dma_start(out=i128_f[:], in_=i128_d[:])
            b_r = cpool.tile([1, G4], F32R, tag="br")
            ones_r = cpool.tile([1, 128], F32R, tag="onesr")
            wx_r = cpool.tile([128, KD * G4], F32R, tag="wxr")
            wh_r = cpool.tile([128, KH * G4], F32R, tag="whr")
            with tc.tile_pool(name="stage", bufs=1) as stage:
                ones_f = stage.tile([1, 128], F32, tag="onesf")
                nc.sync.dma_start(out=ones_f[:], in_=ones_d[:])
                nc.vector.tensor_copy(ones_r[:], ones_f[:])
                b_f = stage.tile([1, G4], F32, tag="bf")
                nc.sync.dma_start(out=b_f[:], in_=b_d[:])
                nc.vector.tensor_copy(b_r[:], b_f[:])
                # Wx: KD k-tiles side by side [128, KD*G4]
                wx_f = stage.tile([128, KD * G4], F32, tag="wxf")
                for k in range(KD):
                    nc.sync.dma_start(out=wx_f[:, k * G4:(k + 1) * G4],
                                      in_=wx_d[k * 128:(k + 1) * 128, :])
                nc.vector.tensor_copy(wx_r[:], wx_f[:])
                # Wh: KH k-tiles side by side [128, KH*G4]
                wh_f = stage.tile([128, KH * G4], F32, tag="whf")
                for k in range(KH):
                    nc.sync.dma_start(out=wh_f[:, k * G4:(k + 1) * G4],
                                      in_=wh_d[k * 128:(k + 1) * 128, :])
                nc.vector.tensor_copy(wh_r[:], wh_f[:])

            # ---------------- phase 1: zx = x @ Wx + b ----------------
            with tc.tile_pool(name="p1sb", bufs=3) as p1, \
                 tc.tile_pool(name="p1ps", bufs=2, space="PSUM") as p1ps, \
                 tc.tile_pool(name="p1pz", bufs=1, space="PSUM") as p1pz:
                for bb in range(BS):
                    for tc0 in range(t_steps // tb):
                        ts0 = tc0 * tb
                        xc = p1.tile([tb, D], F32, tag="xc")
                        nc.sync.dma_start(out=xc[:], in_=x_d[bb, ts0:ts0 + tb, :])
                        ptx = p1ps.tile([128, KD * tb], F32, tag="ptx")
                        xt_r = p1.tile([128, KD * tb], F32R, tag="xtr")
                        for k in range(KD):
                            nc.tensor.transpose(ptx[:, k * tb:(k + 1) * tb],
                                                xc[:, k * 128:(k + 1) * 128],
                                                i128_f[:tb, :tb] if tb < 128 else i128_f[:])
                            nc.vector.tensor_copy(xt_r[:, k * tb:(k + 1) * tb],
                                                  ptx[:, k * tb:(k + 1) * tb])
                        pz1 = p1pz.tile([tb, G4], F32, tag="pz1")
                        for n in range(NG):
                            ns = slice(n * 512, (n + 1) * 512)
                            for k in range(KD):
                                nc.tensor.matmul(
                                    pz1[:, ns], xt_r[:, k * tb:(k + 1) * tb],
                                    wx_r[:, k * G4 + n * 512:k * G4 + (n + 1) * 512],
                                    start=(k == 0), stop=False)
                            nc.tensor.matmul(pz1[:, ns], ones_r[:, :tb],
                                             b_r[:, ns], start=False, stop=True)
                        zxc = p1.tile([tb, G4], F32R, tag="zxc")
                        nc.scalar.activation(zxc[:, 0:1024], pz1[:, 0:1024], AFT.Copy)
                        nc.vector.tensor_copy(zxc[:, 1024:2048], pz1[:, 1024:2048])
                        nc.sync.dma_start(out=zx_d[ts0:ts0 + tb, bb, :], in_=zxc[:])

            # ---------------- phase 2: recurrence ----------------
            with tc.tile_pool(name="zxp", bufs=zx_prefetch) as zxp, \
                 tc.tile_pool(name="st", bufs=2) as st, \
                 tc.tile_pool(name="ew", bufs=3) as ew, \
                 tc.tile_pool(name="pzp", bufs=1, space="PSUM") as pzp, \
                 tc.tile_pool(name="ptp", bufs=2, space="PSUM") as ptp:

                hT_r = st.tile([128, KH * BS], F32R, tag="hT")   # 4 k-tiles [128,16]
                c_t = st.tile([BS, H], F32, tag="c")
                nc.vector.memset(c_t[:], 0.0)
                z0 = ew.tile([128, KH * BS], F32, tag="z0")
                nc.vector.memset(z0[:], 0.0)
                nc.vector.tensor_copy(hT_r[:], z0[:])

                for t in range(t_steps):
                    zxt = zxp.tile([BS, G4], F32R, tag="zx")
                    nc.sync.dma_start(out=zxt[:], in_=zx_d[t, :, :])

                    pz = pzp.tile([BS, G4], F32, tag="pz")
                    for n in range(NG):
                        ns = slice(n * 512, (n + 1) * 512)
                        nc.tensor.matmul(pz[:, ns], i16_r[:], zxt[:, ns],
                                         start=True, stop=False)
                        for k in range(KH):
                            nc.tensor.matmul(
                                pz[:, ns], hT_r[:, k * BS:(k + 1) * BS],
                                wh_r[:, k * G4 + n * 512:k * G4 + (n + 1) * 512],
                                start=False, stop=(k == KH - 1))

                    s = ew.tile([BS, G4], F32, tag="s")
                    nc.scalar.activation(s[:, 0:1024], pz[:, 0:1024], AFT.Sigmoid)
                    nc.scalar.activation(s[:, 1024:1536], pz[:, 1024:1536], AFT.Tanh)
                    nc.scalar.activation(s[:, 1536:2048], pz[:, 1536:2048], AFT.Sigmoid)

                    t1 = ew.tile([BS, H], F32, tag="t1")
                    nc.vector.tensor_mul(t1[:], s[:, 0:512], s[:, 1024:1536])
                    t2 = ew.tile([BS, H], F32, tag="t2")
                    nc.vector.tensor_mul(t2[:], s[:, 512:1024], c_t[:])
                    c_new = st.tile([BS, H], F32, tag="c")
                    nc.vector.tensor_add(c_new[:], t1[:], t2[:])
                    tc_t = ew.tile([BS, H], F32, tag="tc")
                    nc.scalar.activation(tc_t[:], c_new[:], AFT.Tanh)
                    h_new = ew.tile([BS, H], F32, tag="hn")
                    nc.vector.tensor_mul(h_new[:], s[:, 1536:2048], tc_t[:])

                    hT_new = st.tile([128, KH * BS], F32R, tag="hT")
                    pt = ptp.tile([128, KH * BS], F32, tag="pt")
                    for j in range(KH):
                        nc.tensor.transpose(pt[:, j * BS:(j + 1) * BS],
                                            h_new[:, j * 128:(j + 1) * 128],
                                            i16_f[:])
                    for j in range(KH):
                        nc.vector.tensor_copy(hT_new[:, j * BS:(j + 1) * BS],
                                              pt[:, j * BS:(j + 1) * BS])

                    nc.sync.dma_start(out=out_d[:, t, :], in_=h_new[:])
                    if t == t_steps - 1:
                        nc.sync.dma_start(out=hto_d[:], in_=h_new[:])
                        nc.sync.dma_start(out=cto_d[:], in_=c_new[:])

                    hT_r = hT_new
                    c_t = c_new

    nc.compile()
    return nc


_cache = {}


def _get_nc(t_steps=T):
    if t_steps not in _cache:
        _cache[t_steps] = build(t_steps)
    return _cache[t_steps]


def kernel(x, Wx, Wh, b):
    x = np.ascontiguousarray(x, dtype=np.float32)
    Wx = np.ascontiguousarray(Wx, dtype=np.float32)
    Wh = np.ascontiguousarray(Wh, dtype=np.float32)
    b = np.ascontiguousarray(b, dtype=np.float32).reshape(1, G4)
    t_steps = x.shape[1]
    nc = _get_nc(t_steps)

    i16 = np.eye(16, dtype=np.float32)
    i128 = np.eye(128, dtype=np.float32)
    ones = np.ones((1, 128), dtype=np.float32)
    in_maps = [
        {"x": x[c * BS:(c + 1) * BS], "wx": Wx, "wh": Wh, "b": b,
         "i16": i16, "i128": i128, "ones": ones}
        for c in range(NCORES)
    ]
    res = run_bass_kernel_spmd(nc, in_maps, list(range(NCORES)))
    outputs = np.concatenate([res.results[c]["out"] for c in range(NCORES)], axis=0)
    hT = np.concatenate([res.results[c]["hto"] for c in range(NCORES)], axis=0)
    cT = np.concatenate([res.results[c]["cto"] for c in range(NCORES)], axis=0)
    return outputs, hT, cT


# revision 10
# speedup vs baseline: 1.0132x; 1.0132x over previous
"""DynamicRNN (LSTM) Trainium2 kernel.

Strategy (8 NeuronCores, SPMD, zero cross-core communication):
  - Data-parallel: batch B=128 sharded 8 ways (BS=16 per core). LSTM weights
    replicated. The sequential T=512 time loop runs locally per shard.
  - Phase 1: zx = x @ Wx + b for all timesteps, batched as [128-row, 2048]
    matmuls (full PE utilization), written to a DRAM scratch laid out
    [T, BS, 4H] so phase 2 reads one contiguous 128 KB block per step.
  - Phase 2 (recurrence): per step, z = zx_t + h @ Wh via PSUM accumulation:
    an identity matmul initializes each PSUM gate bank with zx_t (emitted one
    step ahead so it fills PE idle time), then 4 k-tile matmuls with
    stationary h^T [128,16] tiles and moving Wh [128,512] chunks.
    Gate columns are host-permuted to (f, i, g, o) so consumers of early
    matmul chunks start sooner. The cell/hidden tail is block-split into
    128-wide hidden blocks so h^T k-tiles become available progressively
    and the next step's matmuls start before the tail finishes.
  - All matmuls run in float32r (rounded fp32): ~bf16 speed with ~1e-4
    relative rounding error. The cell state c stays in full fp32.
"""
import numpy as np
import concourse.bass as bass
import concourse.tile as tile
from concourse import bacc, mybir
from concourse.bass_utils import run_bass_kernel_spmd

B, T, D, H = 128, 512, 256, 512
NCORES = 8
BS = B // NCORES          # 16 batch rows per core
G4 = 4 * H                # 2048
KD = D // 128             # 2 k-tiles for x @ Wx
KH = H // 128             # 4 k-tiles for h @ Wh
NG = G4 // 512            # 4 n-chunks (PSUM bank = 512 fp32)
F32 = mybir.dt.float32
F32R = mybir.dt.float32r
AFT = mybir.ActivationFunctionType
# gate column layout after host permutation: f | i | g | o
SF, SI, SG, SO = (slice(0, 512), slice(512, 1024),
                  slice(1024, 1536), slice(1536, 2048))


def build(t_steps=T, zx_prefetch=8):
    tb = min(128, t_steps)          # phase-1 time-block (rows per chunk)
    assert t_steps % tb == 0
    nc = bacc.Bacc("TRN2", target_bir_lowering=False, debug=False,
                   num_devices=NCORES)

    x_d = nc.dram_tensor("x", [BS, t_steps, D], F32, kind="ExternalInput").ap()
    wx_d = nc.dram_tensor("wx", [D, G4], F32, kind="ExternalInput").ap()
    wh_d = nc.dram_tensor("wh", [H, G4], F32, kind="ExternalInput").ap()
    b_d = nc.dram_tensor("b", [1, G4], F32, kind="ExternalInput").ap()
    i16_d = nc.dram_tensor("i16", [16, 16], F32, kind="ExternalInput").ap()
    i128_d = nc.dram_tensor("i128", [128, 128], F32, kind="ExternalInput").ap()
    ones_d = nc.dram_tensor("ones", [1, 128], F32, kind="ExternalInput").ap()

    out_d = nc.dram_tensor("out", [BS, t_steps, H], F32, kind="ExternalOutput").ap()
    hto_d = nc.dram_tensor("hto", [BS, H], F32, kind="ExternalOutput").ap()
    cto_d = nc.dram_tensor("cto", [BS, H], F32, kind="ExternalOutput").ap()

    zx_d = nc.dram_tensor("zx_scratch", [t_steps, BS, G4], F32R).ap()

    with tile.TileContext(nc) as tc:
        with tc.tile_pool(name="const", bufs=1) as cpool:
            i16_f = cpool.tile([16, 16], F32, tag="i16f")
            nc.sync.dma_start(out=i16_f[:], in_=i16_d[:])
            i16_r = cpool.tile([16, 16], F32R, tag="i16r")
            nc.vector.tensor_copy(i16_r[:], i16_f[:])
            i128_f = cpool.tile([128, 128], F32, tag="i128f")
            nc.sync.dma_start(out=i128_f[:], in_=i128_d[:])
            b_r = cpool.tile([1, G4], F32R, tag="br")
            ones_r = cpool.tile([1, 128], F32R, tag="onesr")
            wx_r = cpool.tile([128, KD * G4], F32R, tag="wxr")
            wh_r = cpool.tile([128, KH * G4], F32R, tag="whr")
            with tc.tile_pool(name="stage", bufs=1) as stage:
                ones_f = stage.tile([1, 128], F32, tag="onesf")
                nc.sync.dma_start(out=ones_f[:], in_=ones_d[:])
                nc.vector.tensor_copy(ones_r[:], ones_f[:])
                b_f = stage.tile([1, G4], F32, tag="bf")
                nc.sync.dma_start(out=b_f[:], in_=b_d[:])
                nc.vector.tensor_copy(b_r[:], b_f[:])
                wx_f = stage.tile([128, KD * G4], F32, tag="wxf")
                for k in range(KD):
                    nc.sync.dma_start(out=wx_f[:, k * G4:(k + 1) * G4],
                                      in_=wx_d[k * 128:(k + 1) * 128, :])
                nc.vector.tensor_copy(wx_r[:], wx_f[:])
                wh_f = stage.tile([128, KH * G4], F32, tag="whf")
                for k in range(KH):
                    nc.sync.dma_start(out=wh_f[:, k * G4:(k + 1) * G4],
                                      in_=wh_d[k * 128:(k + 1) * 128, :])
                nc.vector.tensor_copy(wh_r[:], wh_f[:])

            # ---------------- phase 1: zx = x @ Wx + b ----------------
            with tc.tile_pool(name="p1sb", bufs=3) as p1, \
                 tc.tile_pool(name="p1ps", bufs=2, space="PSUM") as p1ps, \
                 tc.tile_pool(name="p1pz", bufs=2, space="PSUM") as p1pz:
                for bb in range(BS):
                    for tc0 in range(t_steps // tb):
                        ts0 = tc0 * tb
                        xc = p1.tile([tb, D], F32, tag="xc")
                        nc.sync.dma_start(out=xc[:], in_=x_d[bb, ts0:ts0 + tb, :])
                        ptx = p1ps.tile([128, KD * tb], F32, tag="ptx")
                        xt_r = p1.tile([128, KD * tb], F32R, tag="xtr")
                        for k in range(KD):
                            nc.tensor.transpose(ptx[:, k * tb:(k + 1) * tb],
                                                xc[:, k * 128:(k + 1) * 128],
                                                i128_f[:tb, :tb] if tb < 128 else i128_f[:])
                            nc.vector.tensor_copy(xt_r[:, k * tb:(k + 1) * tb],
                                                  ptx[:, k * tb:(k + 1) * tb])
                        zxc = p1.tile([tb, G4], F32R, tag="zxc")
                        for half in range(2):
                            pz1 = p1pz.tile([tb, 1024], F32, tag="pz1",
                                            name=f"pz1_{bb}_{tc0}_{half}")
                            for n2 in range(2):
                                n = half * 2 + n2
                                ps_ = slice(n2 * 512, (n2 + 1) * 512)
                                ns = slice(n * 512, (n + 1) * 512)
                                for k in range(KD):
                                    nc.tensor.matmul(
                                        pz1[:, ps_], xt_r[:, k * tb:(k + 1) * tb],
                                        wx_r[:, k * G4 + n * 512:k * G4 + (n + 1) * 512],
                                        start=(k == 0), stop=False)
                                nc.tensor.matmul(pz1[:, ps_], ones_r[:, :tb],
                                                 b_r[:, ns], start=False, stop=True)
                            if half == 0:
                                nc.scalar.activation(zxc[:, 0:1024], pz1[:], AFT.Copy)
                            else:
                                nc.vector.tensor_copy(zxc[:, 1024:2048], pz1[:])
                        nc.sync.dma_start(out=zx_d[ts0:ts0 + tb, bb, :], in_=zxc[:])

            # ---------------- phase 2: recurrence ----------------
            with tc.tile_pool(name="zxp", bufs=zx_prefetch) as zxp, \
                 tc.tile_pool(name="st", bufs=2) as st, \
                 tc.tile_pool(name="ew", bufs=3) as ew, \
                 tc.tile_pool(name="pzp", bufs=1, space="PSUM") as pzp, \
                 tc.tile_pool(name="ptp", bufs=2, space="PSUM") as ptp:

                # initial state: h^T k-tiles (f32r) and c, all zeros
                hT = [st.tile([128, BS], F32R, tag=f"hT{k}", name=f"hT{k}_init") for k in range(KH)]
                c_t = st.tile([BS, H], F32, tag="c")
                nc.vector.memset(c_t[:], 0.0)
                z0 = ew.tile([128, BS], F32, tag="z0")
                nc.vector.memset(z0[:], 0.0)
                for k in range(KH):
                    nc.vector.tensor_copy(hT[k][:], z0[:])

                pz = pzp.tile([BS, G4], F32, tag="pz")   # 4 banks, one per gate
                zxt = zxp.tile([BS, G4], F32R, tag="zx")
                nc.sync.dma_start(out=zxt[:], in_=zx_d[0, :, :])
                for n in range(NG):
                    nc.tensor.matmul(pz[:, n * 512:(n + 1) * 512], i16_r[:],
                                     zxt[:, n * 512:(n + 1) * 512],
                                     start=True, stop=False)

                for t in range(t_steps):
                    # ---- k-matmuls (accumulate onto zx-initialized banks)
                    for n in range(NG):
                        ns = slice(n * 512, (n + 1) * 512)
                        for k in range(KH):
                            nc.tensor.matmul(
                                pz[:, ns], hT[k][:],
                                wh_r[:, k * G4 + n * 512:k * G4 + (n + 1) * 512],
                                start=False, stop=(k == KH - 1))

                    # ---- gates (whole-gate ACT in chunk completion order)
                    s_f = ew.tile([BS, H], F32, tag="sf")
                    nc.scalar.activation(s_f[:], pz[:, SF], AFT.Sigmoid)
                    s_i = ew.tile([BS, H], F32, tag="si")
                    nc.scalar.activation(s_i[:], pz[:, SI], AFT.Sigmoid)
                    s_g = ew.tile([BS, H], F32, tag="sg")
                    nc.scalar.activation(s_g[:], pz[:, SG], AFT.Tanh)

                    # ---- t2 = f*c on GpSimd (off the critical chain)
                    t2 = ew.tile([BS, H], F32, tag="t2")
                    nc.gpsimd.tensor_mul(t2[:], s_f[:], c_t[:])

                    # ---- block-split tail: 128-wide hidden blocks
                    t1 = ew.tile([BS, H], F32, tag="t1")
                    c_new = st.tile([BS, H], F32, tag="c")
                    tc_t = ew.tile([BS, H], F32, tag="tc")
                    s_o = ew.tile([BS, H], F32, tag="so")
                    h_new = ew.tile([BS, H], F32, tag="hn")
                    for j in range(KH):
                        js = slice(j * 128, (j + 1) * 128)
                        nc.vector.tensor_mul(t1[:, js], s_i[:, js], s_g[:, js])
                    for j in range(KH):
                        js = slice(j * 128, (j + 1) * 128)
                        nc.vector.tensor_add(c_new[:, js], t1[:, js], t2[:, js])
                    for j in range(KH):
                        js = slice(j * 128, (j + 1) * 128)
                        nc.scalar.activation(tc_t[:, js], c_new[:, js], AFT.Tanh)
                        nc.scalar.activation(s_o[:, js], pz[:, 1536 + j * 128:1536 + (j + 1) * 128],
                                             AFT.Sigmoid)
                    for j in range(KH):
                        js = slice(j * 128, (j + 1) * 128)
                        nc.vector.tensor_mul(h_new[:, js], s_o[:, js], tc_t[:, js])

                    # ---- next step's zx PSUM init (fills PE idle time)
                    if t + 1 < t_steps:
                        zxt = zxp.tile([BS, G4], F32R, tag="zx")
                        nc.sync.dma_start(out=zxt[:], in_=zx_d[t + 1, :, :])
                        for n in range(NG):
                            nc.tensor.matmul(pz[:, n * 512:(n + 1) * 512], i16_r[:],
                                             zxt[:, n * 512:(n + 1) * 512],
                                             start=True, stop=False)

                    # ---- h^T for next step: PE transposes + f32r casts
                    hT_new = [st.tile([128, BS], F32R, tag=f"hT{k}", name=f"hT{k}_t{t}") for k in range(KH)]
                    pt = ptp.tile([128, KH * BS], F32, tag="pt")
                    for j in range(KH):
                        nc.tensor.transpose(pt[:, j * BS:(j + 1) * BS],
                                            h_new[:, j * 128:(j + 1) * 128],
                                            i16_f[:])
                        nc.vector.tensor_copy(hT_new[j][:], pt[:, j * BS:(j + 1) * BS])

                    nc.sync.dma_start(out=out_d[:, t, :], in_=h_new[:])
                    if t == t_steps - 1:
                        nc.sync.dma_start(out=hto_d[:], in_=h_new[:])
                        nc.sync.dma_start(out=cto_d[:], in_=c_new[:])

                    hT = hT_new
                    c_t = c_new

    nc.compile()
    return nc


_cache = {}


def _get_nc(t_steps=T):
    if t_steps not in _cache:
        _cache[t_steps] = build(t_steps)
    return _cache[t_steps]


def _permute_gates(w):
    # reference layout i|f|g|o  ->  kernel layout f|i|g|o
    i, f, g, o = np.split(w, 4, axis=-1)
    return np.concatenate([f, i, g, o], axis=-1)


def kernel(x, Wx, Wh, b):
    x = np.ascontiguousarray(x, dtype=np.float32)
    Wx = np.ascontiguousarray(_permute_gates(np.asarray(Wx, dtype=np.float32)))
    Wh = np.ascontiguousarray(_permute_gates(np.asarray(Wh, dtype=np.float32)))
    b = np.ascontiguousarray(_permute_gates(np.asarray(b, dtype=np.float32))).reshape(1, G4)
    t_steps = x.shape[1]
    nc = _get_nc(t_steps)

    i16 = np.eye(16, dtype=np.float32)
    i128 = np.eye(128, dtype=np.float32)
    ones = np.ones((1, 128), dtype=np.float32)
    in_maps = [
        {"x": x[c * BS:(c + 1) * BS], "wx": Wx, "wh": Wh, "b": b,
         "i16": i16, "i128": i128, "ones": ones}
        for c in range(NCORES)
    ]
    res = run_bass_kernel_spmd(nc, in_maps, list(range(NCORES)))
    outputs = np.concatenate([res.results[c]["out"] for c in range(NCORES)], axis=0)
    hT = np.concatenate([res.results[c]["hto"] for c in range(NCORES)], axis=0)
    cT = np.concatenate([res.results[c]["cto"] for c in range(NCORES)], axis=0)
    return outputs, hT, cT


# revision 11
# speedup vs baseline: 1.3986x; 1.3803x over previous
"""DynamicRNN (LSTM) Trainium2 kernel.

Strategy (8 NeuronCores, SPMD, zero cross-core communication):
  - Data-parallel: batch B=128 sharded 8 ways (BS=16 per core). LSTM weights
    replicated. The sequential T=512 time loop runs locally per shard.
  - Phase 1: zx = x @ Wx + b for all timesteps, batched as [128-row, 2048]
    matmuls (full PE utilization), written to a DRAM scratch laid out
    [T, BS, 4H] so phase 2 reads one contiguous 128 KB block per step.
  - Phase 2 (recurrence): per step, z = zx_t + h @ Wh via PSUM accumulation:
    an identity matmul initializes each PSUM gate bank with zx_t (emitted one
    step ahead so it fills PE idle time), then 4 k-tile matmuls with
    stationary h^T [128,16] tiles and moving Wh [128,512] chunks.
    Gate columns are host-permuted to (f, i, g, o) so consumers of early
    matmul chunks start sooner. The cell/hidden tail is block-split into
    128-wide hidden blocks so h^T k-tiles become available progressively
    and the next step's matmuls start before the tail finishes.
  - All matmuls run in float32r (rounded fp32): ~bf16 speed with ~1e-4
    relative rounding error. The cell state c stays in full fp32.
"""
import numpy as np
import concourse.bass as bass
import concourse.tile as tile
from concourse import bacc, mybir
from concourse.bass_utils import run_bass_kernel_spmd

B, T, D, H = 128, 512, 256, 512
NCORES = 8
BS = B // NCORES          # 16 batch rows per core
G4 = 4 * H                # 2048
KD = D // 128             # 2 k-tiles for x @ Wx
KH = H // 128             # 4 k-tiles for h @ Wh
NG = G4 // 512            # 4 n-chunks (PSUM bank = 512 fp32)
F32 = mybir.dt.float32
F32R = mybir.dt.float32r
AFT = mybir.ActivationFunctionType
# gate column layout after host permutation: f | i | g | o
SF, SI, SG, SO = (slice(0, 512), slice(512, 1024),
                  slice(1024, 1536), slice(1536, 2048))


def build(t_steps=T, zx_prefetch=8):
    tb = min(128, t_steps)          # phase-1 time-block (rows per chunk)
    assert t_steps % tb == 0
    nc = bacc.Bacc("TRN2", target_bir_lowering=False, debug=False,
                   num_devices=NCORES)

    x_d = nc.dram_tensor("x", [BS, t_steps, D], F32, kind="ExternalInput").ap()
    wx_d = nc.dram_tensor("wx", [D, G4], F32, kind="ExternalInput").ap()
    wh_d = nc.dram_tensor("wh", [H, G4], F32, kind="ExternalInput").ap()
    b_d = nc.dram_tensor("b", [1, G4], F32, kind="ExternalInput").ap()
    i16_d = nc.dram_tensor("i16", [16, 16], F32, kind="ExternalInput").ap()
    i128_d = nc.dram_tensor("i128", [128, 128], F32, kind="ExternalInput").ap()
    ones_d = nc.dram_tensor("ones", [1, 128], F32, kind="ExternalInput").ap()

    out_d = nc.dram_tensor("out", [BS, t_steps, H], F32, kind="ExternalOutput").ap()
    hto_d = nc.dram_tensor("hto", [BS, H], F32, kind="ExternalOutput").ap()
    cto_d = nc.dram_tensor("cto", [BS, H], F32, kind="ExternalOutput").ap()

    zx_d = nc.dram_tensor("zx_scratch", [t_steps, BS, G4], F32R).ap()

    with tile.TileContext(nc) as tc:
        with tc.tile_pool(name="const", bufs=1) as cpool:
            i16_f = cpool.tile([16, 16], F32, tag="i16f")
            nc.sync.dma_start(out=i16_f[:], in_=i16_d[:])
            i16_r = cpool.tile([16, 16], F32R, tag="i16r")
            nc.vector.tensor_copy(i16_r[:], i16_f[:])
            i128_f = cpool.tile([128, 128], F32, tag="i128f")
            nc.sync.dma_start(out=i128_f[:], in_=i128_d[:])
            b_r = cpool.tile([1, G4], F32R, tag="br")
            ones_r = cpool.tile([1, 128], F32R, tag="onesr")
            wx_r = cpool.tile([128, KD * G4], F32R, tag="wxr")
            wh_r = cpool.tile([128, KH * G4], F32R, tag="whr")
            with tc.tile_pool(name="stage", bufs=1) as stage:
                ones_f = stage.tile([1, 128], F32, tag="onesf")
                nc.sync.dma_start(out=ones_f[:], in_=ones_d[:])
                nc.vector.tensor_copy(ones_r[:], ones_f[:])
                b_f = stage.tile([1, G4], F32, tag="bf")
                nc.sync.dma_start(out=b_f[:], in_=b_d[:])
                nc.vector.tensor_copy(b_r[:], b_f[:])
                wx_f = stage.tile([128, KD * G4], F32, tag="wxf")
                for k in range(KD):
                    nc.sync.dma_start(out=wx_f[:, k * G4:(k + 1) * G4],
                                      in_=wx_d[k * 128:(k + 1) * 128, :])
                nc.vector.tensor_copy(wx_r[:], wx_f[:])
                wh_f = stage.tile([128, KH * G4], F32, tag="whf")
                for k in range(KH):
                    nc.sync.dma_start(out=wh_f[:, k * G4:(k + 1) * G4],
                                      in_=wh_d[k * 128:(k + 1) * 128, :])
                nc.vector.tensor_copy(wh_r[:], wh_f[:])

            # ---------------- phase 1: zx = x @ Wx + b ----------------
            with tc.tile_pool(name="p1sb", bufs=3) as p1, \
                 tc.tile_pool(name="p1ps", bufs=2, space="PSUM") as p1ps, \
                 tc.tile_pool(name="p1pz", bufs=2, space="PSUM") as p1pz:
                for bb in range(BS):
                    for tc0 in range(t_steps // tb):
                        ts0 = tc0 * tb
                        xc = p1.tile([tb, D], F32, tag="xc")
                        nc.sync.dma_start(out=xc[:], in_=x_d[bb, ts0:ts0 + tb, :])
                        ptx = p1ps.tile([128, KD * tb], F32, tag="ptx")
                        xt_r = p1.tile([128, KD * tb], F32R, tag="xtr")
                        for k in range(KD):
                            nc.tensor.transpose(ptx[:, k * tb:(k + 1) * tb],
                                                xc[:, k * 128:(k + 1) * 128],
                                                i128_f[:tb, :tb] if tb < 128 else i128_f[:])
                            nc.vector.tensor_copy(xt_r[:, k * tb:(k + 1) * tb],
                                                  ptx[:, k * tb:(k + 1) * tb])
                        zxc = p1.tile([tb, G4], F32R, tag="zxc")
                        for half in range(2):
                            pz1 = p1pz.tile([tb, 1024], F32, tag="pz1",
                                            name=f"pz1_{bb}_{tc0}_{half}")
                            for n2 in range(2):
                                n = half * 2 + n2
                                ps_ = slice(n2 * 512, (n2 + 1) * 512)
                                ns = slice(n * 512, (n + 1) * 512)
                                for k in range(KD):
                                    nc.tensor.matmul(
                                        pz1[:, ps_], xt_r[:, k * tb:(k + 1) * tb],
                                        wx_r[:, k * G4 + n * 512:k * G4 + (n + 1) * 512],
                                        start=(k == 0), stop=False)
                                nc.tensor.matmul(pz1[:, ps_], ones_r[:, :tb],
                                                 b_r[:, ns], start=False, stop=True)
                            if half == 0:
                                nc.scalar.activation(zxc[:, 0:1024], pz1[:], AFT.Copy)
                            else:
                                nc.vector.tensor_copy(zxc[:, 1024:2048], pz1[:])
                        nc.sync.dma_start(out=zx_d[ts0:ts0 + tb, bb, :], in_=zxc[:])

            # ---------------- phase 2: recurrence ----------------
            with tc.tile_pool(name="zxp", bufs=zx_prefetch) as zxp, \
                 tc.tile_pool(name="st", bufs=2) as st, \
                 tc.tile_pool(name="ew", bufs=3) as ew, \
                 tc.tile_pool(name="pzp", bufs=1, space="PSUM") as pzp, \
                 tc.tile_pool(name="ptp", bufs=2, space="PSUM") as ptp:

                # initial state: h^T k-tiles (f32r) and c, all zeros
                hT = [st.tile([128, BS], F32R, tag=f"hT{k}", name=f"hT{k}_init") for k in range(KH)]
                c_t = st.tile([BS, H], F32, tag="c")
                nc.vector.memset(c_t[:], 0.0)
                z0 = ew.tile([128, BS], F32, tag="z0")
                nc.vector.memset(z0[:], 0.0)
                for k in range(KH):
                    nc.vector.tensor_copy(hT[k][:], z0[:])

                # one PSUM tile per gate so each gate's consumers wait only
                # on that gate's accumulation group (PSUM deps are tile-level)
                pz = [pzp.tile([BS, 512], F32, tag=f"pz{n}", name=f"pz{n}")
                      for n in range(NG)]
                zxt = zxp.tile([BS, G4], F32R, tag="zx")
                nc.sync.dma_start(out=zxt[:], in_=zx_d[0, :, :])
                for n in range(NG):
                    nc.tensor.matmul(pz[n][:], i16_r[:],
                                     zxt[:, n * 512:(n + 1) * 512],
                                     start=True, stop=False)

                for t in range(t_steps):
                    # ---- k-matmuls (accumulate onto zx-initialized banks)
                    for n in range(NG):
                        for k in range(KH):
                            nc.tensor.matmul(
                                pz[n][:], hT[k][:],
                                wh_r[:, k * G4 + n * 512:k * G4 + (n + 1) * 512],
                                start=False, stop=(k == KH - 1))

                    # ---- gates (whole-gate ACT in chunk completion order)
                    s_f = ew.tile([BS, H], F32, tag="sf")
                    nc.scalar.activation(s_f[:], pz[0][:], AFT.Sigmoid)
                    s_i = ew.tile([BS, H], F32, tag="si")
                    nc.scalar.activation(s_i[:], pz[1][:], AFT.Sigmoid)
                    s_g = ew.tile([BS, H], F32, tag="sg")
                    nc.scalar.activation(s_g[:], pz[2][:], AFT.Tanh)

                    # ---- t2 = f*c on GpSimd (off the critical chain)
                    t2 = ew.tile([BS, H], F32, tag="t2")
                    nc.gpsimd.tensor_mul(t2[:], s_f[:], c_t[:])

                    # ---- block-split tail: 128-wide hidden blocks
                    t1 = ew.tile([BS, H], F32, tag="t1")
                    c_new = st.tile([BS, H], F32, tag="c")
                    tc_t = ew.tile([BS, H], F32, tag="tc")
                    s_o = ew.tile([BS, H], F32, tag="so")
                    h_new = ew.tile([BS, H], F32, tag="hn")
                    for j in range(KH):
                        js = slice(j * 128, (j + 1) * 128)
                        nc.vector.tensor_mul(t1[:, js], s_i[:, js], s_g[:, js])
                    for j in range(KH):
                        js = slice(j * 128, (j + 1) * 128)
                        nc.vector.tensor_add(c_new[:, js], t1[:, js], t2[:, js])
                    for j in range(KH):
                        js = slice(j * 128, (j + 1) * 128)
                        nc.scalar.activation(tc_t[:, js], c_new[:, js], AFT.Tanh)
                        nc.scalar.activation(s_o[:, js], pz[3][:, j * 128:(j + 1) * 128],
                                             AFT.Sigmoid)
                    for j in range(KH):
                        js = slice(j * 128, (j + 1) * 128)
                        nc.vector.tensor_mul(h_new[:, js], s_o[:, js], tc_t[:, js])

                    # ---- next step's zx PSUM init (fills PE idle time)
                    if t + 1 < t_steps:
                        zxt = zxp.tile([BS, G4], F32R, tag="zx")
                        nc.sync.dma_start(out=zxt[:], in_=zx_d[t + 1, :, :])
                        for n in range(NG):
                            nc.tensor.matmul(pz[n][:], i16_r[:],
                                             zxt[:, n * 512:(n + 1) * 512],
                                             start=True, stop=False)

                    # ---- h^T for next step: PE transposes + f32r casts
                    hT_new = [st.tile([128, BS], F32R, tag=f"hT{k}", name=f"hT{k}_t{t}") for k in range(KH)]
                    pt = ptp.tile([128, KH * BS], F32, tag="pt")
                    for j in range(KH):
                        nc.tensor.transpose(pt[:, j * BS:(j + 1) * BS],
                                            h_new[:, j * 128:(j + 1) * 128],
                                            i16_f[:])
                        nc.vector.tensor_copy(hT_new[j][:], pt[:, j * BS:(j + 1) * BS])

                    nc.sync.dma_start(out=out_d[:, t, :], in_=h_new[:])
                    if t == t_steps - 1:
                        nc.sync.dma_start(out=hto_d[:], in_=h_new[:])
                        nc.sync.dma_start(out=cto_d[:], in_=c_new[:])

                    hT = hT_new
                    c_t = c_new

    nc.compile()
    return nc


_cache = {}


def _get_nc(t_steps=T):
    if t_steps not in _cache:
        _cache[t_steps] = build(t_steps)
    return _cache[t_steps]


def _permute_gates(w):
    # reference layout i|f|g|o  ->  kernel layout f|i|g|o
    i, f, g, o = np.split(w, 4, axis=-1)
    return np.concatenate([f, i, g, o], axis=-1)


def kernel(x, Wx, Wh, b):
    x = np.ascontiguousarray(x, dtype=np.float32)
    Wx = np.ascontiguousarray(_permute_gates(np.asarray(Wx, dtype=np.float32)))
    Wh = np.ascontiguousarray(_permute_gates(np.asarray(Wh, dtype=np.float32)))
    b = np.ascontiguousarray(_permute_gates(np.asarray(b, dtype=np.float32))).reshape(1, G4)
    t_steps = x.shape[1]
    nc = _get_nc(t_steps)

    i16 = np.eye(16, dtype=np.float32)
    i128 = np.eye(128, dtype=np.float32)
    ones = np.ones((1, 128), dtype=np.float32)
    in_maps = [
        {"x": x[c * BS:(c + 1) * BS], "wx": Wx, "wh": Wh, "b": b,
         "i16": i16, "i128": i128, "ones": ones}
        for c in range(NCORES)
    ]
    res = run_bass_kernel_spmd(nc, in_maps, list(range(NCORES)))
    outputs = np.concatenate([res.results[c]["out"] for c in range(NCORES)], axis=0)
    hT = np.concatenate([res.results[c]["hto"] for c in range(NCORES)], axis=0)
    cT = np.concatenate([res.results[c]["cto"] for c in range(NCORES)], axis=0)
    return outputs, hT, cT
